# revision 36
# baseline (speedup 1.0000x reference)
"""Trainium2 Bass kernel for AtomPositionGather (segment reduce over sorted atom->residue map).

8-core SPMD data-parallel over atoms. Host shards at residue-aligned atom
boundaries and renumbers residues per core into "virtual" ids such that each
640-atom window owns <=128 residues starting in it -> one compile-time-uniform
schedule works for every core. Device does all segment reductions (feature
sums via one-hot fp32r matmuls into PSUM windows, encoded count columns,
telescoped last-CA/last-CB position columns) plus the per-residue nonlinear
epilogue (means, mask, 3x3 frames). Host unshards by row permutation.
"""

import os
import numpy as np

import concourse.bass as bass
import concourse.bacc as bacc
import concourse.mybir as mybir
from concourse.tile import TileContext
from concourse.bass_utils import run_bass_kernel_spmd

P = 128
APAD = 64000          # padded atoms per core
WATOMS = 640          # atoms per window
NWIN = APAD // WATOMS # 100 windows
RV = NWIN * P         # 12800 virtual residues per core
TPW = WATOMS // P     # 5 tiles per window
CHUNK_W = 4           # windows per feature DMA chunk
TPC = TPW * CHUNK_W   # 20 tiles per chunk
NCHUNK = NWIN // CHUNK_W
NCORES = 8
A_TOT = 500_000
R_TOT = 62_500
HID = 128
ID_N, ID_CA, ID_C, ID_CB = 0, 1, 2, 4
ENC = 4096.0

f32 = mybir.dt.float32
f32r = mybir.dt.float32r
bf16 = mybir.dt.bfloat16
i32 = mybir.dt.int32
A = mybir.AluOpType
AF = mybir.ActivationFunctionType

_COMPILED = {}


def _build_nc():
    nc = bacc.Bacc()
    feats = nc.dram_tensor("feats", [APAD, HID], f32, kind="ExternalInput")
    cols = nc.dram_tensor("cols", [NCHUNK, P, TPC, 16], f32, kind="ExternalInput")
    out_feat = nc.dram_tensor("out_feat", [RV, HID], f32, kind="ExternalOutput")
    out_small = nc.dram_tensor("out_small", [RV, 16], f32, kind="ExternalOutput")

    with TileContext(nc) as tc:
        with (
            tc.tile_pool(name="const", bufs=1) as const_pool,
            tc.tile_pool(name="stage", bufs=1) as stage_pool,
            tc.tile_pool(name="fchunk", bufs=3) as f_pool,
            tc.tile_pool(name="cchunk", bufs=3) as c_pool,
            tc.tile_pool(name="ohp", bufs=4) as oh_pool,
            tc.tile_pool(name="rhsp", bufs=4) as rhs_pool,
            tc.tile_pool(name="psumw", bufs=4, space="PSUM") as psum_pool,
        ):
            # residue-id iota replicated across partitions: [p, v] = v
            iotaRV = const_pool.tile([P, RV + P], f32)
            nc.gpsimd.iota(iotaRV[:], pattern=[[1, RV + P]], base=0,
                           channel_multiplier=0,
                           allow_small_or_imprecise_dtypes=True)

            stage_feat = stage_pool.tile([P, NWIN * HID], f32)
            stage_small = stage_pool.tile([P, NWIN * 16], f32)   # w-major, ch-minor

            psum_tiles = {}
            NT = NWIN * TPW

            def epilogue(w):
                pw = psum_tiles.pop(w)
                nc.scalar.activation(
                    out=stage_feat[:, w * HID:(w + 1) * HID], in_=pw[:, 0:HID],
                    func=AF.Copy)
                nc.vector.tensor_copy(
                    out=stage_small[:, w * 16:(w + 1) * 16],
                    in_=pw[:, HID:HID + 16])

            for chunk in range(NCHUNK):
                fch = f_pool.tile([P, TPC * HID], f32, tag="fch")
                nc.sync.dma_start(
                    out=fch[:, 0:TPC * HID].rearrange("p (t f) -> p t f", f=HID),
                    in_=feats[:].rearrange("(c t p) f -> c p t f", p=P, t=TPC)[chunk],
                )
                cch0 = c_pool.tile([P, TPC * 16], f32, tag="cch0")
                nc.sync.dma_start(
                    out=cch0[:],
                    in_=cols[:][chunk].rearrange("p t c -> p (t c)"),
                )
                # route through DVE so per-tile consumers depend on DVE program
                # order instead of DMA semaphores (avoids sync-wait overflow)
                cch = c_pool.tile([P, TPC * 16], f32, tag="cch")
                nc.vector.tensor_copy(out=cch[:], in_=cch0[:])
                def onehot_pair(w, t):
                    """straddle-tile one-hots vs window w (per-tile build)"""
                    ohca = oh_pool.tile([P, P], f32, tag="ohca")
                    nc.vector.tensor_tensor(
                        out=ohca[:], in0=iotaRV[:, w * P:(w + 1) * P],
                        in1=cch[:, t * 16:t * 16 + 1].to_broadcast([P, P]),
                        op=A.is_equal)
                    ohall = oh_pool.tile([P, P], f32, tag="ohall")
                    nc.vector.tensor_tensor(
                        out=ohall[:], in0=iotaRV[:, w * P:(w + 1) * P],
                        in1=cch[:, t * 16 + 1:t * 16 + 2].to_broadcast([P, P]),
                        op=A.is_equal)
                    return ohca, ohall

                def onehot_window(w, wl):
                    """all TPW tiles' one-hots for window w in two DVE ops"""
                    t0 = wl * TPW
                    segs = cch[:, t0 * 16:(t0 + TPW) * 16]
                    wca = oh_pool.tile([P, TPW * P], f32, tag="wca")
                    nc.vector.tensor_tensor(
                        out=wca[:].rearrange("p (t f) -> p t f", f=P),
                        in0=iotaRV[:, w * P:(w + 1) * P]
                            .unsqueeze(1).to_broadcast([P, TPW, P]),
                        in1=segs.rearrange("p (t c) -> p t c", c=16)[:, :, 0:1]
                            .to_broadcast([P, TPW, P]),
                        op=A.is_equal)
                    wall = oh_pool.tile([P, TPW * P], f32, tag="wall")
                    nc.vector.tensor_tensor(
                        out=wall[:].rearrange("p (t f) -> p t f", f=P),
                        in0=iotaRV[:, w * P:(w + 1) * P]
                            .unsqueeze(1).to_broadcast([P, TPW, P]),
                        in1=segs.rearrange("p (t c) -> p t c", c=16)[:, :, 1:2]
                            .to_broadcast([P, TPW, P]),
                        op=A.is_equal)
                    return wca, wall

                win_oh = {}

                for t in range(TPC):
                    gt = chunk * TPC + t
                    w = gt // TPW
                    first_of_w = (gt % TPW == 0)
                    last_mm = (gt == NT - 1)
                    if first_of_w:
                        pw = psum_pool.tile([P, HID + 16], f32, tag="pw")
                        psum_tiles[w] = pw
                        nc.vector.memset(pw[:], 0.0)
                    else:
                        pw = psum_tiles[w]

                    fsrc = fch[:, t * HID:(t + 1) * HID]
                    rhs_b = cch[:, t * 16:(t + 1) * 16]
                    if first_of_w:
                        win_oh[w] = onehot_window(w, t // TPW)
                    wca, wall = win_oh[w]
                    tl0 = (t % TPW) * P
                    nc.tensor.matmul(
                        out=pw[:, 0:HID], lhsT=wca[:, tl0:tl0 + P], rhs=fsrc,
                        start=False, stop=last_mm, skip_group_check=True)
                    nc.tensor.matmul(
                        out=pw[:, HID:HID + 16], lhsT=wall[:, tl0:tl0 + P],
                        rhs=rhs_b,
                        start=False, stop=last_mm, skip_group_check=True)

                    if first_of_w and w > 0:
                        ohca2, ohall2 = onehot_pair(w - 1, t)
                        pprev = psum_tiles[w - 1]
                        nc.tensor.matmul(
                            out=pprev[:, 0:HID], lhsT=ohca2[:],
                            rhs=fsrc, start=False, stop=True,
                            skip_group_check=True)
                        nc.tensor.matmul(
                            out=pprev[:, HID:HID + 16],
                            lhsT=ohall2[:],
                            rhs=rhs_b, start=False, stop=True,
                            skip_group_check=True)
                        epilogue(w - 1)
            epilogue(NWIN - 1)

            # ---------------- bulk per-residue math ----------------
            with tc.tile_pool(name="bulk", bufs=1) as bulk_pool:
                Bp = bulk_pool.tile([P, 16 * NWIN], f32)   # channel-major planes
                # deinterleave [p,(w c)] -> [p,(c w)]
                nc.vector.tensor_copy(
                    out=Bp[:].rearrange("p (c w) -> p c w", w=NWIN),
                    in_=stage_small[:].rearrange("p (w c) -> p c w", c=16))

                B = bulk_pool.tile([P, 26 * NWIN], f32)

                def bt(idx):
                    return B[:, idx * NWIN:(idx + 1) * NWIN]

                def plane(c):
                    return Bp[:, c * NWIN:(c + 1) * NWIN]

                ca_cnt = plane(2)
                enc2, enc3 = plane(3), plane(4)
                cax, cay, caz = plane(5), plane(6), plane(7)
                cbx, cby, cbz = plane(8), plane(9), plane(10)

                # feature means: *= 1/max(ca_cnt,1)
                recip = bt(0)
                nc.vector.tensor_scalar_max(recip, ca_cnt, 1.0)
                nc.vector.reciprocal(recip, recip)
                nc.vector.tensor_tensor(
                    out=stage_feat[:].rearrange("p (w f) -> p w f", f=HID),
                    in0=stage_feat[:].rearrange("p (w f) -> p w f", f=HID),
                    in1=recip.unsqueeze(2).to_broadcast([P, NWIN, HID]),
                    op=A.mult)

                # decode counts
                nN, cnt, nC, nCB = bt(1), bt(2), bt(3), bt(4)
                enc_i = bt(5)
                nc.vector.tensor_copy(out=enc_i.bitcast(i32), in_=enc2)
                nc.vector.tensor_scalar(
                    nN.bitcast(i32), enc_i.bitcast(i32), 12, None,
                    A.logical_shift_right)
                nc.vector.tensor_scalar(
                    cnt.bitcast(i32), enc_i.bitcast(i32), 4095, None,
                    A.bitwise_and)
                nc.vector.tensor_copy(out=enc_i.bitcast(i32), in_=enc3)
                nc.vector.tensor_scalar(
                    nCB.bitcast(i32), enc_i.bitcast(i32), 12, None,
                    A.logical_shift_right)
                nc.vector.tensor_scalar(
                    nC.bitcast(i32), enc_i.bitcast(i32), 4095, None,
                    A.bitwise_and)
                for x in (nN, cnt, nC, nCB):
                    nc.vector.tensor_copy(out=x, in_=x.bitcast(i32))

                # mask
                mask, tmp = bt(6), bt(7)
                nc.vector.tensor_scalar(mask, cnt, 3.0, None, A.is_ge)
                nc.vector.tensor_scalar(tmp, nN, 0.5, None, A.is_ge)
                nc.vector.tensor_tensor(out=mask, in0=mask, in1=tmp, op=A.mult)
                nc.vector.tensor_scalar(tmp, ca_cnt, 0.5, None, A.is_ge)
                nc.vector.tensor_tensor(out=mask, in0=mask, in1=tmp, op=A.mult)
                nc.vector.tensor_scalar(tmp, nC, 0.5, None, A.is_ge)
                nc.vector.tensor_tensor(out=mask, in0=mask, in1=tmp, op=A.mult)

                # pos_CB fallback -> pos_CA where no CB atom
                nocb = bt(8)
                nc.vector.tensor_scalar(nocb, nCB, 0.5, None, A.is_lt)
                for csrc, cdst in ((cax, cbx), (cay, cby), (caz, cbz)):
                    nc.vector.tensor_tensor(out=tmp, in0=nocb, in1=csrc, op=A.mult)
                    nc.vector.tensor_tensor(out=cdst, in0=cdst, in1=tmp, op=A.add)

                # frames
                e1x, e1y, e1z = bt(9), bt(10), bt(11)
                nc.vector.tensor_tensor(out=e1x, in0=cbx, in1=cax, op=A.subtract)
                nc.vector.tensor_tensor(out=e1y, in0=cby, in1=cay, op=A.subtract)
                nc.vector.tensor_tensor(out=e1z, in0=cbz, in1=caz, op=A.subtract)
                n1sq, valid, n1 = bt(12), bt(13), bt(14)
                nc.vector.tensor_tensor(out=n1sq, in0=e1x, in1=e1x, op=A.mult)
                nc.vector.tensor_tensor(out=tmp, in0=e1y, in1=e1y, op=A.mult)
                nc.vector.tensor_tensor(out=n1sq, in0=n1sq, in1=tmp, op=A.add)
                nc.vector.tensor_tensor(out=tmp, in0=e1z, in1=e1z, op=A.mult)
                nc.vector.tensor_tensor(out=n1sq, in0=n1sq, in1=tmp, op=A.add)
                nc.vector.tensor_scalar(valid, n1sq, 1e-12, None, A.is_gt)
                nc.scalar.activation(out=n1, in_=n1sq, func=AF.Sqrt)
                nc.vector.tensor_scalar_max(n1, n1, 1e-12)
                nc.vector.reciprocal(n1, n1)
                for e in (e1x, e1y, e1z):
                    nc.vector.tensor_tensor(out=e, in0=e, in1=n1, op=A.mult)
                n2asq, usey, noty = bt(15), bt(16), bt(17)
                nc.vector.tensor_tensor(out=n2asq, in0=e1x, in1=e1x, op=A.mult)
                nc.vector.tensor_tensor(out=tmp, in0=e1y, in1=e1y, op=A.mult)
                nc.vector.tensor_tensor(out=n2asq, in0=n2asq, in1=tmp, op=A.add)
                nc.vector.tensor_scalar(usey, n2asq, 1e-12, None, A.is_lt)
                nc.vector.tensor_scalar(noty, usey, -1.0, 1.0, A.mult, A.add)
                # e2 = usey ? (-e1z,0,e1x) : (e1y,-e1x,0)
                e2x, e2y, e2z, t2 = bt(18), bt(19), bt(20), bt(21)
                nc.vector.tensor_tensor(out=e2x, in0=noty, in1=e1y, op=A.mult)
                nc.vector.tensor_tensor(out=t2, in0=usey, in1=e1z, op=A.mult)
                nc.vector.tensor_tensor(out=e2x, in0=e2x, in1=t2, op=A.subtract)
                nc.vector.tensor_tensor(out=e2y, in0=noty, in1=e1x, op=A.mult)
                nc.vector.tensor_scalar(e2y, e2y, -1.0, None, A.mult)
                nc.vector.tensor_tensor(out=e2z, in0=usey, in1=e1x, op=A.mult)
                n2sq = bt(22)
                nc.vector.tensor_tensor(out=n2sq, in0=e2x, in1=e2x, op=A.mult)
                nc.vector.tensor_tensor(out=t2, in0=e2y, in1=e2y, op=A.mult)
                nc.vector.tensor_tensor(out=n2sq, in0=n2sq, in1=t2, op=A.add)
                nc.vector.tensor_tensor(out=t2, in0=e2z, in1=e2z, op=A.mult)
                nc.vector.tensor_tensor(out=n2sq, in0=n2sq, in1=t2, op=A.add)
                nc.vector.tensor_scalar(t2, n2sq, 1e-12, None, A.is_gt)
                nc.vector.tensor_tensor(out=valid, in0=valid, in1=t2, op=A.mult)
                n2 = bt(23)
                nc.scalar.activation(out=n2, in_=n2sq, func=AF.Sqrt)
                nc.vector.tensor_scalar_max(n2, n2, 1e-12)
                nc.vector.reciprocal(n2, n2)
                for e in (e2x, e2y, e2z):
                    nc.vector.tensor_tensor(out=e, in0=e, in1=n2, op=A.mult)
                e3x, e3y, e3z = bt(24), bt(25), n1sq
                nc.vector.tensor_tensor(out=e3x, in0=e1y, in1=e2z, op=A.mult)
                nc.vector.tensor_tensor(out=t2, in0=e1z, in1=e2y, op=A.mult)
                nc.vector.tensor_tensor(out=e3x, in0=e3x, in1=t2, op=A.subtract)
                nc.vector.tensor_tensor(out=e3y, in0=e1z, in1=e2x, op=A.mult)
                nc.vector.tensor_tensor(out=t2, in0=e1x, in1=e2z, op=A.mult)
                nc.vector.tensor_tensor(out=e3y, in0=e3y, in1=t2, op=A.subtract)
                nc.vector.tensor_tensor(out=e3z, in0=e1x, in1=e2y, op=A.mult)
                nc.vector.tensor_tensor(out=t2, in0=e1y, in1=e2x, op=A.mult)
                nc.vector.tensor_tensor(out=e3z, in0=e3z, in1=t2, op=A.subtract)

                # assemble outputs (channel-major planes, then interleave + DMA)
                OutP = bulk_pool.tile([P, 16 * NWIN], f32)

                def outp(c):
                    return OutP[:, c * NWIN:(c + 1) * NWIN]

                nc.vector.tensor_copy(out=outp(0), in_=cax)
                nc.vector.tensor_copy(out=outp(1), in_=cay)
                nc.vector.tensor_copy(out=outp(2), in_=caz)
                nc.vector.tensor_copy(out=outp(3), in_=cbx)
                nc.vector.tensor_copy(out=outp(4), in_=cby)
                nc.vector.tensor_copy(out=outp(5), in_=cbz)
                frames = [e1x, e1y, e1z, e2x, e2y, e2z, e3x, e3y, e3z]
                notv = bt(5)  # reuse
                nc.vector.tensor_scalar(notv, valid, -1.0, 1.0, A.mult, A.add)
                for j in range(3):
                    for i in range(3):
                        src = frames[j * 3 + i]
                        dst = outp(6 + i * 3 + j)
                        nc.vector.tensor_tensor(out=dst, in0=src, in1=valid,
                                                op=A.mult)
                        if i == j:
                            nc.vector.tensor_tensor(out=dst, in0=dst, in1=notv,
                                                    op=A.add)
                nc.vector.tensor_copy(out=outp(15), in_=mask)

                OutS = bulk_pool.tile([P, NWIN * 16], f32)
                nc.vector.tensor_copy(
                    out=OutS[:].rearrange("p (w c) -> p c w", c=16),
                    in_=OutP[:].rearrange("p (c w) -> p c w", w=NWIN))

                nc.sync.dma_start(
                    out=out_feat[:].rearrange("(w p) f -> p w f", p=P),
                    in_=stage_feat[:].rearrange("p (w f) -> p w f", f=HID))
                nc.sync.dma_start(
                    out=out_small[:].rearrange("(w p) c -> p w c", p=P),
                    in_=OutS[:].rearrange("p (w c) -> p w c", c=16))
    nc.finalize()
    return nc


def _host_prep(node_features, node_positions, atom_type, atom2residue):
    seg = np.asarray(atom2residue, dtype=np.int64)
    atype = np.asarray(atom_type, dtype=np.int64)
    pos = np.asarray(node_positions, dtype=np.float32)
    r_edges = [round(k * R_TOT / NCORES) for k in range(NCORES + 1)]
    a_edges = np.searchsorted(seg, r_edges).astype(np.int64)
    a_edges[0], a_edges[-1] = 0, A_TOT

    in_maps, metas = [], []
    for k in range(NCORES):
        a0, a1 = int(a_edges[k]), int(a_edges[k + 1])
        assert a1 - a0 <= APAD, f"core {k}: {a1 - a0} atoms > APAD"
        s = min(a0, A_TOT - APAD)
        sl = slice(s, s + APAD)
        segk = seg[sl]
        typk = atype[sl]
        posk = pos[sl]
        off = np.arange(APAD)
        real = (off >= a0 - s) & (off < a1 - s)

        segr = np.where(real, segk, -1)
        uids, first = np.unique(segr, return_index=True)
        if uids[0] == -1:
            uids, first = uids[1:], first[1:]
        win = first // WATOMS
        rank = np.zeros_like(win)
        for w in np.unique(win):
            m = win == w
            nw = int(m.sum())
            assert nw <= P, f"core {k} window {w}: {nw} residues > 128"
            rank[m] = np.arange(nw)
        vid = win * P + rank
        vmap = np.full(R_TOT, -1, dtype=np.int64)
        vmap[uids] = vid
        segv = np.where(real, vmap[np.clip(segr, 0, R_TOT - 1)], -1)

        ca = real & (typk == ID_CA)
        cb = real & (typk == ID_CB)
        isn = real & (typk == ID_N)
        isc = real & (typk == ID_C)

        def tele(mask):
            d = np.zeros((APAD, 3), np.float32)
            idx = np.nonzero(mask)[0]
            if len(idx):
                same = np.zeros(len(idx), bool)
                same[1:] = segk[idx[1:]] == segk[idx[:-1]]
                prev = np.zeros((len(idx), 3), np.float32)
                prev[1:] = posk[idx[:-1]]
                d[idx] = posk[idx] - np.where(same[:, None], prev, 0.0)
            return d

        dca = tele(ca)
        dcb = tele(cb)

        cols = np.zeros((APAD, 16), np.float32)
        segv_f = np.where(segv >= 0, segv, -100000).astype(np.float32)
        cols[:, 0] = np.where(ca, segv_f, -100000.0)   # CA-masked seg
        cols[:, 1] = segv_f                            # unmasked seg
        cols[:, 2] = ca
        cols[:, 3] = real.astype(np.float32) + ENC * isn
        cols[:, 4] = isc.astype(np.float32) + ENC * cb
        cols[:, 5:8] = dca
        cols[:, 8:11] = dcb
        cols_sw = np.ascontiguousarray(
            cols.reshape(NCHUNK, TPC, P, 16).transpose(0, 2, 1, 3))

        in_maps.append({
            "feats": np.ascontiguousarray(node_features[sl]),
            "cols": cols_sw,
        })
        metas.append({"uids": uids, "vid": vid})
    return in_maps, metas


def kernel(node_features, node_positions, atom_type, atom2residue,
           num_residues=R_TOT):
    node_features = np.asarray(node_features, dtype=np.float32)
    node_positions = np.asarray(node_positions, dtype=np.float32)
    atom_type_np = np.asarray(atom_type, dtype=np.int32)
    atom2residue_np = np.asarray(atom2residue, dtype=np.int32)

    in_maps, metas = _host_prep(node_features, node_positions,
                                atom_type_np, atom2residue_np)
    if "nc" not in _COMPILED:
        _COMPILED["nc"] = _build_nc()
    nc = _COMPILED["nc"]

    trace = bool(int(os.environ.get("KERNEL_TRACE", "0")))
    res = run_bass_kernel_spmd(nc, in_maps, core_ids=list(range(NCORES)),
                               trace=trace)
    if res.exec_time_ns is not None:
        print(f"HW exec time: {res.exec_time_ns} ns")
        _COMPILED["exec_time_ns"] = res.exec_time_ns

    R = R_TOT
    res_feat = np.zeros((R, HID), np.float32)
    pos_ca = np.zeros((R, 3), np.float32)
    pos_cb = np.zeros((R, 3), np.float32)
    frames = np.tile(np.eye(3, dtype=np.float32)[None], (R, 1, 1))
    maskf = np.zeros(R, np.float32)
    for k in range(NCORES):
        m = metas[k]
        out_f = res.results[k]["out_feat"]
        out_s = res.results[k]["out_small"]
        uids, vid = m["uids"], m["vid"]
        res_feat[uids] = out_f[vid]
        sm = out_s[vid]
        pos_ca[uids] = sm[:, 0:3]
        pos_cb[uids] = sm[:, 3:6]
        frames[uids] = sm[:, 6:15].reshape(-1, 3, 3)
        maskf[uids] = sm[:, 15]
    frames[R - 1] = np.eye(3, dtype=np.float32)
    residue_mask = maskf > 0.5
    return (res_feat, pos_ca, pos_cb, frames, atom2residue_np, residue_mask)


# revision 37
# speedup vs baseline: 1.0516x; 1.0516x over previous
"""Trainium2 Bass kernel for AtomPositionGather (segment reduce over sorted atom->residue map).

8-core SPMD data-parallel over atoms. Host shards at residue-aligned atom
boundaries and renumbers residues per core into "virtual" ids such that each
640-atom window owns <=128 residues starting in it -> one compile-time-uniform
schedule works for every core. Device does all segment reductions (feature
sums via one-hot fp32r matmuls into PSUM windows, encoded count columns,
telescoped last-CA/last-CB position columns) plus the per-residue nonlinear
epilogue (means, mask, 3x3 frames). Host unshards by row permutation.
"""

import os
import numpy as np

import concourse.bass as bass
import concourse.bacc as bacc
import concourse.mybir as mybir
from concourse.tile import TileContext
from concourse.bass_utils import run_bass_kernel_spmd

P = 128
APAD = 64000          # padded atoms per core
WATOMS = 640          # atoms per window
NWIN = APAD // WATOMS # 100 windows
RV = NWIN * P         # 12800 virtual residues per core
TPW = WATOMS // P     # 5 tiles per window
CHUNK_W = 4           # windows per feature DMA chunk
TPC = TPW * CHUNK_W   # 20 tiles per chunk
NCHUNK = NWIN // CHUNK_W
NCORES = 8
A_TOT = 500_000
R_TOT = 62_500
HID = 128
ID_N, ID_CA, ID_C, ID_CB = 0, 1, 2, 4
ENC = 4096.0

f32 = mybir.dt.float32
f32r = mybir.dt.float32r
bf16 = mybir.dt.bfloat16
i32 = mybir.dt.int32
A = mybir.AluOpType
AF = mybir.ActivationFunctionType

_COMPILED = {}


def _build_nc():
    nc = bacc.Bacc()
    feats = nc.dram_tensor("feats", [APAD, HID], f32, kind="ExternalInput")
    cols = nc.dram_tensor("cols", [NCHUNK, P, TPC, 16], f32, kind="ExternalInput")
    out_feat = nc.dram_tensor("out_feat", [RV, HID], f32, kind="ExternalOutput")
    out_small = nc.dram_tensor("out_small", [RV, 16], f32, kind="ExternalOutput")

    with TileContext(nc) as tc:
        with (
            tc.tile_pool(name="const", bufs=1) as const_pool,
            tc.tile_pool(name="stage", bufs=1) as stage_pool,
            tc.tile_pool(name="fchunk", bufs=3) as f_pool,
            tc.tile_pool(name="cchunk", bufs=3) as c_pool,
            tc.tile_pool(name="ohp", bufs=4) as oh_pool,
            tc.tile_pool(name="rhsp", bufs=4) as rhs_pool,
            tc.tile_pool(name="psumw", bufs=4, space="PSUM") as psum_pool,
        ):
            # residue-id iota replicated across partitions: [p, v] = v
            iotaRV = const_pool.tile([P, RV + P], f32)
            nc.gpsimd.iota(iotaRV[:], pattern=[[1, RV + P]], base=0,
                           channel_multiplier=0,
                           allow_small_or_imprecise_dtypes=True)

            stage_feat = stage_pool.tile([P, NWIN * HID], f32)
            stage_small = stage_pool.tile([P, NWIN * 16], f32)   # w-major, ch-minor

            psum_tiles = {}
            NT = NWIN * TPW

            def epilogue(w):
                pw = psum_tiles.pop(w)
                nc.scalar.activation(
                    out=stage_feat[:, w * HID:(w + 1) * HID], in_=pw[:, 0:HID],
                    func=AF.Copy)
                nc.scalar.activation(
                    out=stage_small[:, w * 16:(w + 1) * 16],
                    in_=pw[:, HID:HID + 16], func=AF.Copy)

            for chunk in range(NCHUNK):
                fch = f_pool.tile([P, TPC * HID], f32, tag="fch")
                nc.sync.dma_start(
                    out=fch[:, 0:TPC * HID].rearrange("p (t f) -> p t f", f=HID),
                    in_=feats[:].rearrange("(c t p) f -> c p t f", p=P, t=TPC)[chunk],
                )
                cch0 = c_pool.tile([P, TPC * 16], f32, tag="cch0")
                nc.sync.dma_start(
                    out=cch0[:],
                    in_=cols[:][chunk].rearrange("p t c -> p (t c)"),
                )
                # route through DVE so per-tile consumers depend on DVE program
                # order instead of DMA semaphores (avoids sync-wait overflow)
                cch = c_pool.tile([P, TPC * 16], f32, tag="cch")
                nc.vector.tensor_copy(out=cch[:], in_=cch0[:])
                def onehot_pair(w, t):
                    """straddle-tile one-hots vs window w (per-tile build)"""
                    ohca = oh_pool.tile([P, P], f32, tag="ohca")
                    nc.vector.tensor_tensor(
                        out=ohca[:], in0=iotaRV[:, w * P:(w + 1) * P],
                        in1=cch[:, t * 16:t * 16 + 1].to_broadcast([P, P]),
                        op=A.is_equal)
                    ohall = oh_pool.tile([P, P], f32, tag="ohall")
                    nc.vector.tensor_tensor(
                        out=ohall[:], in0=iotaRV[:, w * P:(w + 1) * P],
                        in1=cch[:, t * 16 + 1:t * 16 + 2].to_broadcast([P, P]),
                        op=A.is_equal)
                    return ohca, ohall

                def onehot_window(w, wl):
                    """all TPW tiles' one-hots for window w in two DVE ops"""
                    t0 = wl * TPW
                    segs = cch[:, t0 * 16:(t0 + TPW) * 16]
                    wca = oh_pool.tile([P, TPW * P], f32, tag="wca")
                    nc.vector.tensor_tensor(
                        out=wca[:].rearrange("p (t f) -> p t f", f=P),
                        in0=iotaRV[:, w * P:(w + 1) * P]
                            .unsqueeze(1).to_broadcast([P, TPW, P]),
                        in1=segs.rearrange("p (t c) -> p t c", c=16)[:, :, 0:1]
                            .to_broadcast([P, TPW, P]),
                        op=A.is_equal)
                    wall = oh_pool.tile([P, TPW * P], f32, tag="wall")
                    nc.vector.tensor_tensor(
                        out=wall[:].rearrange("p (t f) -> p t f", f=P),
                        in0=iotaRV[:, w * P:(w + 1) * P]
                            .unsqueeze(1).to_broadcast([P, TPW, P]),
                        in1=segs.rearrange("p (t c) -> p t c", c=16)[:, :, 1:2]
                            .to_broadcast([P, TPW, P]),
                        op=A.is_equal)
                    return wca, wall

                win_oh = {}

                for t in range(TPC):
                    gt = chunk * TPC + t
                    w = gt // TPW
                    first_of_w = (gt % TPW == 0)
                    last_mm = (gt == NT - 1)
                    if first_of_w:
                        pw = psum_pool.tile([P, HID + 16], f32, tag="pw")
                        psum_tiles[w] = pw
                        nc.vector.memset(pw[:], 0.0)
                    else:
                        pw = psum_tiles[w]

                    fsrc = fch[:, t * HID:(t + 1) * HID]
                    rhs_b = cch[:, t * 16:(t + 1) * 16]
                    if first_of_w:
                        win_oh[w] = onehot_window(w, t // TPW)
                    wca, wall = win_oh[w]
                    tl0 = (t % TPW) * P
                    nc.tensor.matmul(
                        out=pw[:, 0:HID], lhsT=wca[:, tl0:tl0 + P], rhs=fsrc,
                        start=False, stop=last_mm, skip_group_check=True)
                    nc.tensor.matmul(
                        out=pw[:, HID:HID + 16], lhsT=wall[:, tl0:tl0 + P],
                        rhs=rhs_b,
                        start=False, stop=last_mm, skip_group_check=True)

                    if first_of_w and w > 0:
                        ohca2, ohall2 = onehot_pair(w - 1, t)
                        pprev = psum_tiles[w - 1]
                        nc.tensor.matmul(
                            out=pprev[:, 0:HID], lhsT=ohca2[:],
                            rhs=fsrc, start=False, stop=True,
                            skip_group_check=True)
                        nc.tensor.matmul(
                            out=pprev[:, HID:HID + 16],
                            lhsT=ohall2[:],
                            rhs=rhs_b, start=False, stop=True,
                            skip_group_check=True)
                        epilogue(w - 1)
            epilogue(NWIN - 1)

            # ---------------- bulk per-residue math ----------------
            with tc.tile_pool(name="bulk", bufs=1) as bulk_pool:
                Bp = bulk_pool.tile([P, 16 * NWIN], f32)   # channel-major planes
                # deinterleave [p,(w c)] -> [p,(c w)]
                nc.vector.tensor_copy(
                    out=Bp[:].rearrange("p (c w) -> p c w", w=NWIN),
                    in_=stage_small[:].rearrange("p (w c) -> p c w", c=16))

                B = bulk_pool.tile([P, 26 * NWIN], f32)

                def bt(idx):
                    return B[:, idx * NWIN:(idx + 1) * NWIN]

                def plane(c):
                    return Bp[:, c * NWIN:(c + 1) * NWIN]

                ca_cnt = plane(2)
                enc2, enc3 = plane(3), plane(4)
                cax, cay, caz = plane(5), plane(6), plane(7)
                cbx, cby, cbz = plane(8), plane(9), plane(10)

                # feature means: *= 1/max(ca_cnt,1)
                recip = bt(0)
                nc.vector.tensor_scalar_max(recip, ca_cnt, 1.0)
                nc.vector.reciprocal(recip, recip)
                nc.vector.tensor_tensor(
                    out=stage_feat[:].rearrange("p (w f) -> p w f", f=HID),
                    in0=stage_feat[:].rearrange("p (w f) -> p w f", f=HID),
                    in1=recip.unsqueeze(2).to_broadcast([P, NWIN, HID]),
                    op=A.mult)

                # decode counts
                nN, cnt, nC, nCB = bt(1), bt(2), bt(3), bt(4)
                enc_i = bt(5)
                nc.vector.tensor_copy(out=enc_i.bitcast(i32), in_=enc2)
                nc.vector.tensor_scalar(
                    nN.bitcast(i32), enc_i.bitcast(i32), 12, None,
                    A.logical_shift_right)
                nc.vector.tensor_scalar(
                    cnt.bitcast(i32), enc_i.bitcast(i32), 4095, None,
                    A.bitwise_and)
                nc.vector.tensor_copy(out=enc_i.bitcast(i32), in_=enc3)
                nc.vector.tensor_scalar(
                    nCB.bitcast(i32), enc_i.bitcast(i32), 12, None,
                    A.logical_shift_right)
                nc.vector.tensor_scalar(
                    nC.bitcast(i32), enc_i.bitcast(i32), 4095, None,
                    A.bitwise_and)
                for x in (nN, cnt, nC, nCB):
                    nc.vector.tensor_copy(out=x, in_=x.bitcast(i32))

                # mask
                mask, tmp = bt(6), bt(7)
                nc.vector.tensor_scalar(mask, cnt, 3.0, None, A.is_ge)
                nc.vector.tensor_scalar(tmp, nN, 0.5, None, A.is_ge)
                nc.vector.tensor_tensor(out=mask, in0=mask, in1=tmp, op=A.mult)
                nc.vector.tensor_scalar(tmp, ca_cnt, 0.5, None, A.is_ge)
                nc.vector.tensor_tensor(out=mask, in0=mask, in1=tmp, op=A.mult)
                nc.vector.tensor_scalar(tmp, nC, 0.5, None, A.is_ge)
                nc.vector.tensor_tensor(out=mask, in0=mask, in1=tmp, op=A.mult)

                # pos_CB fallback -> pos_CA where no CB atom
                nocb = bt(8)
                nc.vector.tensor_scalar(nocb, nCB, 0.5, None, A.is_lt)
                for csrc, cdst in ((cax, cbx), (cay, cby), (caz, cbz)):
                    nc.vector.tensor_tensor(out=tmp, in0=nocb, in1=csrc, op=A.mult)
                    nc.vector.tensor_tensor(out=cdst, in0=cdst, in1=tmp, op=A.add)

                # frames
                e1x, e1y, e1z = bt(9), bt(10), bt(11)
                nc.vector.tensor_tensor(out=e1x, in0=cbx, in1=cax, op=A.subtract)
                nc.vector.tensor_tensor(out=e1y, in0=cby, in1=cay, op=A.subtract)
                nc.vector.tensor_tensor(out=e1z, in0=cbz, in1=caz, op=A.subtract)
                n1sq, valid, n1 = bt(12), bt(13), bt(14)
                nc.vector.tensor_tensor(out=n1sq, in0=e1x, in1=e1x, op=A.mult)
                nc.vector.tensor_tensor(out=tmp, in0=e1y, in1=e1y, op=A.mult)
                nc.vector.tensor_tensor(out=n1sq, in0=n1sq, in1=tmp, op=A.add)
                nc.vector.tensor_tensor(out=tmp, in0=e1z, in1=e1z, op=A.mult)
                nc.vector.tensor_tensor(out=n1sq, in0=n1sq, in1=tmp, op=A.add)
                nc.vector.tensor_scalar(valid, n1sq, 1e-12, None, A.is_gt)
                nc.scalar.activation(out=n1, in_=n1sq, func=AF.Sqrt)
                nc.vector.tensor_scalar_max(n1, n1, 1e-12)
                nc.vector.reciprocal(n1, n1)
                for e in (e1x, e1y, e1z):
                    nc.vector.tensor_tensor(out=e, in0=e, in1=n1, op=A.mult)
                n2asq, usey, noty = bt(15), bt(16), bt(17)
                nc.vector.tensor_tensor(out=n2asq, in0=e1x, in1=e1x, op=A.mult)
                nc.vector.tensor_tensor(out=tmp, in0=e1y, in1=e1y, op=A.mult)
                nc.vector.tensor_tensor(out=n2asq, in0=n2asq, in1=tmp, op=A.add)
                nc.vector.tensor_scalar(usey, n2asq, 1e-12, None, A.is_lt)
                nc.vector.tensor_scalar(noty, usey, -1.0, 1.0, A.mult, A.add)
                # e2 = usey ? (-e1z,0,e1x) : (e1y,-e1x,0)
                e2x, e2y, e2z, t2 = bt(18), bt(19), bt(20), bt(21)
                nc.vector.tensor_tensor(out=e2x, in0=noty, in1=e1y, op=A.mult)
                nc.vector.tensor_tensor(out=t2, in0=usey, in1=e1z, op=A.mult)
                nc.vector.tensor_tensor(out=e2x, in0=e2x, in1=t2, op=A.subtract)
                nc.vector.tensor_tensor(out=e2y, in0=noty, in1=e1x, op=A.mult)
                nc.vector.tensor_scalar(e2y, e2y, -1.0, None, A.mult)
                nc.vector.tensor_tensor(out=e2z, in0=usey, in1=e1x, op=A.mult)
                n2sq = bt(22)
                nc.vector.tensor_tensor(out=n2sq, in0=e2x, in1=e2x, op=A.mult)
                nc.vector.tensor_tensor(out=t2, in0=e2y, in1=e2y, op=A.mult)
                nc.vector.tensor_tensor(out=n2sq, in0=n2sq, in1=t2, op=A.add)
                nc.vector.tensor_tensor(out=t2, in0=e2z, in1=e2z, op=A.mult)
                nc.vector.tensor_tensor(out=n2sq, in0=n2sq, in1=t2, op=A.add)
                nc.vector.tensor_scalar(t2, n2sq, 1e-12, None, A.is_gt)
                nc.vector.tensor_tensor(out=valid, in0=valid, in1=t2, op=A.mult)
                n2 = bt(23)
                nc.scalar.activation(out=n2, in_=n2sq, func=AF.Sqrt)
                nc.vector.tensor_scalar_max(n2, n2, 1e-12)
                nc.vector.reciprocal(n2, n2)
                for e in (e2x, e2y, e2z):
                    nc.vector.tensor_tensor(out=e, in0=e, in1=n2, op=A.mult)
                e3x, e3y, e3z = bt(24), bt(25), n1sq
                nc.vector.tensor_tensor(out=e3x, in0=e1y, in1=e2z, op=A.mult)
                nc.vector.tensor_tensor(out=t2, in0=e1z, in1=e2y, op=A.mult)
                nc.vector.tensor_tensor(out=e3x, in0=e3x, in1=t2, op=A.subtract)
                nc.vector.tensor_tensor(out=e3y, in0=e1z, in1=e2x, op=A.mult)
                nc.vector.tensor_tensor(out=t2, in0=e1x, in1=e2z, op=A.mult)
                nc.vector.tensor_tensor(out=e3y, in0=e3y, in1=t2, op=A.subtract)
                nc.vector.tensor_tensor(out=e3z, in0=e1x, in1=e2y, op=A.mult)
                nc.vector.tensor_tensor(out=t2, in0=e1y, in1=e2x, op=A.mult)
                nc.vector.tensor_tensor(out=e3z, in0=e3z, in1=t2, op=A.subtract)

                # assemble outputs (channel-major planes, then interleave + DMA)
                OutP = bulk_pool.tile([P, 16 * NWIN], f32)

                def outp(c):
                    return OutP[:, c * NWIN:(c + 1) * NWIN]

                nc.vector.tensor_copy(out=outp(0), in_=cax)
                nc.vector.tensor_copy(out=outp(1), in_=cay)
                nc.vector.tensor_copy(out=outp(2), in_=caz)
                nc.vector.tensor_copy(out=outp(3), in_=cbx)
                nc.vector.tensor_copy(out=outp(4), in_=cby)
                nc.vector.tensor_copy(out=outp(5), in_=cbz)
                frames = [e1x, e1y, e1z, e2x, e2y, e2z, e3x, e3y, e3z]
                notv = bt(5)  # reuse
                nc.vector.tensor_scalar(notv, valid, -1.0, 1.0, A.mult, A.add)
                for j in range(3):
                    for i in range(3):
                        src = frames[j * 3 + i]
                        dst = outp(6 + i * 3 + j)
                        nc.vector.tensor_tensor(out=dst, in0=src, in1=valid,
                                                op=A.mult)
                        if i == j:
                            nc.vector.tensor_tensor(out=dst, in0=dst, in1=notv,
                                                    op=A.add)
                nc.vector.tensor_copy(out=outp(15), in_=mask)

                OutS = bulk_pool.tile([P, NWIN * 16], f32)
                nc.vector.tensor_copy(
                    out=OutS[:].rearrange("p (w c) -> p c w", c=16),
                    in_=OutP[:].rearrange("p (c w) -> p c w", w=NWIN))

                nc.sync.dma_start(
                    out=out_feat[:].rearrange("(w p) f -> p w f", p=P),
                    in_=stage_feat[:].rearrange("p (w f) -> p w f", f=HID))
                nc.sync.dma_start(
                    out=out_small[:].rearrange("(w p) c -> p w c", p=P),
                    in_=OutS[:].rearrange("p (w c) -> p w c", c=16))
    nc.finalize()
    return nc


def _host_prep(node_features, node_positions, atom_type, atom2residue):
    seg = np.asarray(atom2residue, dtype=np.int64)
    atype = np.asarray(atom_type, dtype=np.int64)
    pos = np.asarray(node_positions, dtype=np.float32)
    r_edges = [round(k * R_TOT / NCORES) for k in range(NCORES + 1)]
    a_edges = np.searchsorted(seg, r_edges).astype(np.int64)
    a_edges[0], a_edges[-1] = 0, A_TOT

    in_maps, metas = [], []
    for k in range(NCORES):
        a0, a1 = int(a_edges[k]), int(a_edges[k + 1])
        assert a1 - a0 <= APAD, f"core {k}: {a1 - a0} atoms > APAD"
        s = min(a0, A_TOT - APAD)
        sl = slice(s, s + APAD)
        segk = seg[sl]
        typk = atype[sl]
        posk = pos[sl]
        off = np.arange(APAD)
        real = (off >= a0 - s) & (off < a1 - s)

        segr = np.where(real, segk, -1)
        uids, first = np.unique(segr, return_index=True)
        if uids[0] == -1:
            uids, first = uids[1:], first[1:]
        win = first // WATOMS
        rank = np.zeros_like(win)
        for w in np.unique(win):
            m = win == w
            nw = int(m.sum())
            assert nw <= P, f"core {k} window {w}: {nw} residues > 128"
            rank[m] = np.arange(nw)
        vid = win * P + rank
        vmap = np.full(R_TOT, -1, dtype=np.int64)
        vmap[uids] = vid
        segv = np.where(real, vmap[np.clip(segr, 0, R_TOT - 1)], -1)

        ca = real & (typk == ID_CA)
        cb = real & (typk == ID_CB)
        isn = real & (typk == ID_N)
        isc = real & (typk == ID_C)

        def tele(mask):
            d = np.zeros((APAD, 3), np.float32)
            idx = np.nonzero(mask)[0]
            if len(idx):
                same = np.zeros(len(idx), bool)
                same[1:] = segk[idx[1:]] == segk[idx[:-1]]
                prev = np.zeros((len(idx), 3), np.float32)
                prev[1:] = posk[idx[:-1]]
                d[idx] = posk[idx] - np.where(same[:, None], prev, 0.0)
            return d

        dca = tele(ca)
        dcb = tele(cb)

        cols = np.zeros((APAD, 16), np.float32)
        segv_f = np.where(segv >= 0, segv, -100000).astype(np.float32)
        cols[:, 0] = np.where(ca, segv_f, -100000.0)   # CA-masked seg
        cols[:, 1] = segv_f                            # unmasked seg
        cols[:, 2] = ca
        cols[:, 3] = real.astype(np.float32) + ENC * isn
        cols[:, 4] = isc.astype(np.float32) + ENC * cb
        cols[:, 5:8] = dca
        cols[:, 8:11] = dcb
        cols_sw = np.ascontiguousarray(
            cols.reshape(NCHUNK, TPC, P, 16).transpose(0, 2, 1, 3))

        in_maps.append({
            "feats": np.ascontiguousarray(node_features[sl]),
            "cols": cols_sw,
        })
        metas.append({"uids": uids, "vid": vid})
    return in_maps, metas


def kernel(node_features, node_positions, atom_type, atom2residue,
           num_residues=R_TOT):
    node_features = np.asarray(node_features, dtype=np.float32)
    node_positions = np.asarray(node_positions, dtype=np.float32)
    atom_type_np = np.asarray(atom_type, dtype=np.int32)
    atom2residue_np = np.asarray(atom2residue, dtype=np.int32)

    in_maps, metas = _host_prep(node_features, node_positions,
                                atom_type_np, atom2residue_np)
    if "nc" not in _COMPILED:
        _COMPILED["nc"] = _build_nc()
    nc = _COMPILED["nc"]

    trace = bool(int(os.environ.get("KERNEL_TRACE", "0")))
    res = run_bass_kernel_spmd(nc, in_maps, core_ids=list(range(NCORES)),
                               trace=trace)
    if res.exec_time_ns is not None:
        print(f"HW exec time: {res.exec_time_ns} ns")
        _COMPILED["exec_time_ns"] = res.exec_time_ns

    R = R_TOT
    res_feat = np.zeros((R, HID), np.float32)
    pos_ca = np.zeros((R, 3), np.float32)
    pos_cb = np.zeros((R, 3), np.float32)
    frames = np.tile(np.eye(3, dtype=np.float32)[None], (R, 1, 1))
    maskf = np.zeros(R, np.float32)
    for k in range(NCORES):
        m = metas[k]
        out_f = res.results[k]["out_feat"]
        out_s = res.results[k]["out_small"]
        uids, vid = m["uids"], m["vid"]
        res_feat[uids] = out_f[vid]
        sm = out_s[vid]
        pos_ca[uids] = sm[:, 0:3]
        pos_cb[uids] = sm[:, 3:6]
        frames[uids] = sm[:, 6:15].reshape(-1, 3, 3)
        maskf[uids] = sm[:, 15]
    frames[R - 1] = np.eye(3, dtype=np.float32)
    residue_mask = maskf > 0.5
    return (res_feat, pos_ca, pos_cb, frames, atom2residue_np, residue_mask)


# revision 38
# speedup vs baseline: 1.1681x; 1.1108x over previous
"""Trainium2 Bass kernel for AtomPositionGather (segment reduce over sorted atom->residue map).

8-core SPMD data-parallel over atoms. Host shards at residue-aligned atom
boundaries and renumbers residues per core into "virtual" ids such that each
640-atom window owns <=128 residues starting in it -> one compile-time-uniform
schedule works for every core. Device does all segment reductions (feature
sums via one-hot fp32r matmuls into PSUM windows, encoded count columns,
telescoped last-CA/last-CB position columns) plus the per-residue nonlinear
epilogue (means, mask, 3x3 frames). Host unshards by row permutation.
"""

import os
import numpy as np

import concourse.bass as bass
import concourse.bacc as bacc
import concourse.mybir as mybir
from concourse.tile import TileContext
from concourse.bass_utils import run_bass_kernel_spmd

P = 128
APAD = 64000          # padded atoms per core
WATOMS = 640          # atoms per window
NWIN = APAD // WATOMS # 100 windows
RV = NWIN * P         # 12800 virtual residues per core
TPW = WATOMS // P     # 5 tiles per window
CHUNK_W = 4           # windows per feature DMA chunk
TPC = TPW * CHUNK_W   # 20 tiles per chunk
NCHUNK = NWIN // CHUNK_W
NCORES = 8
A_TOT = 500_000
R_TOT = 62_500
HID = 128
ID_N, ID_CA, ID_C, ID_CB = 0, 1, 2, 4
ENC = 4096.0

f32 = mybir.dt.float32
f32r = mybir.dt.float32r
bf16 = mybir.dt.bfloat16
i32 = mybir.dt.int32
A = mybir.AluOpType
AF = mybir.ActivationFunctionType

_COMPILED = {}


def _build_nc():
    nc = bacc.Bacc()
    feats = nc.dram_tensor("feats", [APAD, HID], f32, kind="ExternalInput")
    cols = nc.dram_tensor("cols", [NCHUNK, P, TPC, 16], f32, kind="ExternalInput")
    out_feat = nc.dram_tensor("out_feat", [RV, HID], f32, kind="ExternalOutput")
    out_small = nc.dram_tensor("out_small", [RV, 16], f32, kind="ExternalOutput")

    with TileContext(nc) as tc:
        with (
            tc.tile_pool(name="const", bufs=1) as const_pool,
            tc.tile_pool(name="stage", bufs=1) as stage_pool,
            tc.tile_pool(name="fchunk", bufs=3) as f_pool,
            tc.tile_pool(name="cchunk", bufs=3) as c_pool,
            tc.tile_pool(name="ohp", bufs=4) as oh_pool,
            tc.tile_pool(name="rhsp", bufs=4) as rhs_pool,
            tc.tile_pool(name="psumw", bufs=4, space="PSUM") as psum_pool,
        ):
            # residue-id iota replicated across partitions: [p, v] = v
            iotaRV = const_pool.tile([P, RV + P], f32)
            nc.gpsimd.iota(iotaRV[:], pattern=[[1, RV + P]], base=0,
                           channel_multiplier=0,
                           allow_small_or_imprecise_dtypes=True)

            zeros144 = const_pool.tile([P, HID + 16], f32)
            nc.vector.memset(zeros144[:], 0.0)
            stage_feat = stage_pool.tile([P, NWIN * HID], f32)
            stage_small = stage_pool.tile([P, NWIN * 16], f32)   # w-major, ch-minor

            psum_tiles = {}
            NT = NWIN * TPW

            def epilogue(w):
                pw = psum_tiles.pop(w)
                nc.scalar.activation(
                    out=stage_feat[:, w * HID:(w + 1) * HID], in_=pw[:, 0:HID],
                    func=AF.Copy)
                nc.scalar.activation(
                    out=stage_small[:, w * 16:(w + 1) * 16],
                    in_=pw[:, HID:HID + 16], func=AF.Copy)

            for chunk in range(NCHUNK):
                fch = f_pool.tile([P, TPC * HID], f32, tag="fch")
                nc.sync.dma_start(
                    out=fch[:, 0:TPC * HID].rearrange("p (t f) -> p t f", f=HID),
                    in_=feats[:].rearrange("(c t p) f -> c p t f", p=P, t=TPC)[chunk],
                )
                cch0 = c_pool.tile([P, TPC * 16], f32, tag="cch0")
                nc.sync.dma_start(
                    out=cch0[:],
                    in_=cols[:][chunk].rearrange("p t c -> p (t c)"),
                )
                # route through DVE so per-tile consumers depend on DVE program
                # order instead of DMA semaphores (avoids sync-wait overflow)
                cch = c_pool.tile([P, TPC * 16], f32, tag="cch")
                nc.vector.tensor_copy(out=cch[:], in_=cch0[:])
                def onehot_pair(w, t):
                    """straddle-tile one-hots vs window w (per-tile build)"""
                    ohca = oh_pool.tile([P, P], f32, tag="ohca")
                    nc.vector.tensor_tensor(
                        out=ohca[:], in0=iotaRV[:, w * P:(w + 1) * P],
                        in1=cch[:, t * 16:t * 16 + 1].to_broadcast([P, P]),
                        op=A.is_equal)
                    ohall = oh_pool.tile([P, P], f32, tag="ohall")
                    nc.vector.tensor_tensor(
                        out=ohall[:], in0=iotaRV[:, w * P:(w + 1) * P],
                        in1=cch[:, t * 16 + 1:t * 16 + 2].to_broadcast([P, P]),
                        op=A.is_equal)
                    return ohca, ohall

                def onehot_window(w, wl):
                    """all TPW tiles' one-hots for window w in two DVE ops"""
                    t0 = wl * TPW
                    segs = cch[:, t0 * 16:(t0 + TPW) * 16]
                    wca = oh_pool.tile([P, TPW * P], f32, tag="wca")
                    nc.vector.tensor_tensor(
                        out=wca[:].rearrange("p (t f) -> p t f", f=P),
                        in0=iotaRV[:, w * P:(w + 1) * P]
                            .unsqueeze(1).to_broadcast([P, TPW, P]),
                        in1=segs.rearrange("p (t c) -> p t c", c=16)[:, :, 0:1]
                            .to_broadcast([P, TPW, P]),
                        op=A.is_equal)
                    wall = oh_pool.tile([P, TPW * P], f32, tag="wall")
                    nc.vector.tensor_tensor(
                        out=wall[:].rearrange("p (t f) -> p t f", f=P),
                        in0=iotaRV[:, w * P:(w + 1) * P]
                            .unsqueeze(1).to_broadcast([P, TPW, P]),
                        in1=segs.rearrange("p (t c) -> p t c", c=16)[:, :, 1:2]
                            .to_broadcast([P, TPW, P]),
                        op=A.is_equal)
                    return wca, wall

                win_oh = {}

                for t in range(TPC):
                    gt = chunk * TPC + t
                    w = gt // TPW
                    first_of_w = (gt % TPW == 0)
                    last_mm = (gt == NT - 1)
                    if first_of_w:
                        pw = psum_pool.tile([P, HID + 16], f32, tag="pw")
                        psum_tiles[w] = pw
                        nc.scalar.activation(out=pw[:], in_=zeros144[:],
                                             func=AF.Copy)
                    else:
                        pw = psum_tiles[w]

                    fsrc = fch[:, t * HID:(t + 1) * HID]
                    rhs_b = cch[:, t * 16:(t + 1) * 16]
                    if first_of_w:
                        win_oh[w] = onehot_window(w, t // TPW)
                    wca, wall = win_oh[w]
                    tl0 = (t % TPW) * P
                    nc.tensor.matmul(
                        out=pw[:, 0:HID], lhsT=wca[:, tl0:tl0 + P], rhs=fsrc,
                        start=False, stop=last_mm, skip_group_check=True)
                    nc.tensor.matmul(
                        out=pw[:, HID:HID + 16], lhsT=wall[:, tl0:tl0 + P],
                        rhs=rhs_b,
                        start=False, stop=last_mm, skip_group_check=True)

                    if first_of_w and w > 0:
                        ohca2, ohall2 = onehot_pair(w - 1, t)
                        pprev = psum_tiles[w - 1]
                        nc.tensor.matmul(
                            out=pprev[:, 0:HID], lhsT=ohca2[:],
                            rhs=fsrc, start=False, stop=True,
                            skip_group_check=True)
                        nc.tensor.matmul(
                            out=pprev[:, HID:HID + 16],
                            lhsT=ohall2[:],
                            rhs=rhs_b, start=False, stop=True,
                            skip_group_check=True)
                        epilogue(w - 1)
            epilogue(NWIN - 1)

            # ---------------- bulk per-residue math ----------------
            with tc.tile_pool(name="bulk", bufs=1) as bulk_pool:
                Bp = bulk_pool.tile([P, 16 * NWIN], f32)   # channel-major planes
                # deinterleave [p,(w c)] -> [p,(c w)]
                nc.vector.tensor_copy(
                    out=Bp[:].rearrange("p (c w) -> p c w", w=NWIN),
                    in_=stage_small[:].rearrange("p (w c) -> p c w", c=16))

                B = bulk_pool.tile([P, 26 * NWIN], f32)

                def bt(idx):
                    return B[:, idx * NWIN:(idx + 1) * NWIN]

                def plane(c):
                    return Bp[:, c * NWIN:(c + 1) * NWIN]

                ca_cnt = plane(2)
                enc2, enc3 = plane(3), plane(4)
                cax, cay, caz = plane(5), plane(6), plane(7)
                cbx, cby, cbz = plane(8), plane(9), plane(10)

                # feature means: *= 1/max(ca_cnt,1)
                recip = bt(0)
                nc.vector.tensor_scalar_max(recip, ca_cnt, 1.0)
                nc.vector.reciprocal(recip, recip)
                nc.vector.tensor_tensor(
                    out=stage_feat[:].rearrange("p (w f) -> p w f", f=HID),
                    in0=stage_feat[:].rearrange("p (w f) -> p w f", f=HID),
                    in1=recip.unsqueeze(2).to_broadcast([P, NWIN, HID]),
                    op=A.mult)

                # decode counts
                nN, cnt, nC, nCB = bt(1), bt(2), bt(3), bt(4)
                enc_i = bt(5)
                nc.vector.tensor_copy(out=enc_i.bitcast(i32), in_=enc2)
                nc.vector.tensor_scalar(
                    nN.bitcast(i32), enc_i.bitcast(i32), 12, None,
                    A.logical_shift_right)
                nc.vector.tensor_scalar(
                    cnt.bitcast(i32), enc_i.bitcast(i32), 4095, None,
                    A.bitwise_and)
                nc.vector.tensor_copy(out=enc_i.bitcast(i32), in_=enc3)
                nc.vector.tensor_scalar(
                    nCB.bitcast(i32), enc_i.bitcast(i32), 12, None,
                    A.logical_shift_right)
                nc.vector.tensor_scalar(
                    nC.bitcast(i32), enc_i.bitcast(i32), 4095, None,
                    A.bitwise_and)
                for x in (nN, cnt, nC, nCB):
                    nc.vector.tensor_copy(out=x, in_=x.bitcast(i32))

                # mask
                mask, tmp = bt(6), bt(7)
                nc.vector.tensor_scalar(mask, cnt, 3.0, None, A.is_ge)
                nc.vector.tensor_scalar(tmp, nN, 0.5, None, A.is_ge)
                nc.vector.tensor_tensor(out=mask, in0=mask, in1=tmp, op=A.mult)
                nc.vector.tensor_scalar(tmp, ca_cnt, 0.5, None, A.is_ge)
                nc.vector.tensor_tensor(out=mask, in0=mask, in1=tmp, op=A.mult)
                nc.vector.tensor_scalar(tmp, nC, 0.5, None, A.is_ge)
                nc.vector.tensor_tensor(out=mask, in0=mask, in1=tmp, op=A.mult)

                # pos_CB fallback -> pos_CA where no CB atom
                nocb = bt(8)
                nc.vector.tensor_scalar(nocb, nCB, 0.5, None, A.is_lt)
                for csrc, cdst in ((cax, cbx), (cay, cby), (caz, cbz)):
                    nc.vector.tensor_tensor(out=tmp, in0=nocb, in1=csrc, op=A.mult)
                    nc.vector.tensor_tensor(out=cdst, in0=cdst, in1=tmp, op=A.add)

                # frames
                e1x, e1y, e1z = bt(9), bt(10), bt(11)
                nc.vector.tensor_tensor(out=e1x, in0=cbx, in1=cax, op=A.subtract)
                nc.vector.tensor_tensor(out=e1y, in0=cby, in1=cay, op=A.subtract)
                nc.vector.tensor_tensor(out=e1z, in0=cbz, in1=caz, op=A.subtract)
                n1sq, valid, n1 = bt(12), bt(13), bt(14)
                nc.vector.tensor_tensor(out=n1sq, in0=e1x, in1=e1x, op=A.mult)
                nc.vector.tensor_tensor(out=tmp, in0=e1y, in1=e1y, op=A.mult)
                nc.vector.tensor_tensor(out=n1sq, in0=n1sq, in1=tmp, op=A.add)
                nc.vector.tensor_tensor(out=tmp, in0=e1z, in1=e1z, op=A.mult)
                nc.vector.tensor_tensor(out=n1sq, in0=n1sq, in1=tmp, op=A.add)
                nc.vector.tensor_scalar(valid, n1sq, 1e-12, None, A.is_gt)
                nc.scalar.activation(out=n1, in_=n1sq, func=AF.Sqrt)
                nc.vector.tensor_scalar_max(n1, n1, 1e-12)
                nc.vector.reciprocal(n1, n1)
                for e in (e1x, e1y, e1z):
                    nc.vector.tensor_tensor(out=e, in0=e, in1=n1, op=A.mult)
                n2asq, usey, noty = bt(15), bt(16), bt(17)
                nc.vector.tensor_tensor(out=n2asq, in0=e1x, in1=e1x, op=A.mult)
                nc.vector.tensor_tensor(out=tmp, in0=e1y, in1=e1y, op=A.mult)
                nc.vector.tensor_tensor(out=n2asq, in0=n2asq, in1=tmp, op=A.add)
                nc.vector.tensor_scalar(usey, n2asq, 1e-12, None, A.is_lt)
                nc.vector.tensor_scalar(noty, usey, -1.0, 1.0, A.mult, A.add)
                # e2 = usey ? (-e1z,0,e1x) : (e1y,-e1x,0)
                e2x, e2y, e2z, t2 = bt(18), bt(19), bt(20), bt(21)
                nc.vector.tensor_tensor(out=e2x, in0=noty, in1=e1y, op=A.mult)
                nc.vector.tensor_tensor(out=t2, in0=usey, in1=e1z, op=A.mult)
                nc.vector.tensor_tensor(out=e2x, in0=e2x, in1=t2, op=A.subtract)
                nc.vector.tensor_tensor(out=e2y, in0=noty, in1=e1x, op=A.mult)
                nc.vector.tensor_scalar(e2y, e2y, -1.0, None, A.mult)
                nc.vector.tensor_tensor(out=e2z, in0=usey, in1=e1x, op=A.mult)
                n2sq = bt(22)
                nc.vector.tensor_tensor(out=n2sq, in0=e2x, in1=e2x, op=A.mult)
                nc.vector.tensor_tensor(out=t2, in0=e2y, in1=e2y, op=A.mult)
                nc.vector.tensor_tensor(out=n2sq, in0=n2sq, in1=t2, op=A.add)
                nc.vector.tensor_tensor(out=t2, in0=e2z, in1=e2z, op=A.mult)
                nc.vector.tensor_tensor(out=n2sq, in0=n2sq, in1=t2, op=A.add)
                nc.vector.tensor_scalar(t2, n2sq, 1e-12, None, A.is_gt)
                nc.vector.tensor_tensor(out=valid, in0=valid, in1=t2, op=A.mult)
                n2 = bt(23)
                nc.scalar.activation(out=n2, in_=n2sq, func=AF.Sqrt)
                nc.vector.tensor_scalar_max(n2, n2, 1e-12)
                nc.vector.reciprocal(n2, n2)
                for e in (e2x, e2y, e2z):
                    nc.vector.tensor_tensor(out=e, in0=e, in1=n2, op=A.mult)
                e3x, e3y, e3z = bt(24), bt(25), n1sq
                nc.vector.tensor_tensor(out=e3x, in0=e1y, in1=e2z, op=A.mult)
                nc.vector.tensor_tensor(out=t2, in0=e1z, in1=e2y, op=A.mult)
                nc.vector.tensor_tensor(out=e3x, in0=e3x, in1=t2, op=A.subtract)
                nc.vector.tensor_tensor(out=e3y, in0=e1z, in1=e2x, op=A.mult)
                nc.vector.tensor_tensor(out=t2, in0=e1x, in1=e2z, op=A.mult)
                nc.vector.tensor_tensor(out=e3y, in0=e3y, in1=t2, op=A.subtract)
                nc.vector.tensor_tensor(out=e3z, in0=e1x, in1=e2y, op=A.mult)
                nc.vector.tensor_tensor(out=t2, in0=e1y, in1=e2x, op=A.mult)
                nc.vector.tensor_tensor(out=e3z, in0=e3z, in1=t2, op=A.subtract)

                # assemble outputs (channel-major planes, then interleave + DMA)
                OutP = bulk_pool.tile([P, 16 * NWIN], f32)

                def outp(c):
                    return OutP[:, c * NWIN:(c + 1) * NWIN]

                nc.vector.tensor_copy(out=outp(0), in_=cax)
                nc.vector.tensor_copy(out=outp(1), in_=cay)
                nc.vector.tensor_copy(out=outp(2), in_=caz)
                nc.vector.tensor_copy(out=outp(3), in_=cbx)
                nc.vector.tensor_copy(out=outp(4), in_=cby)
                nc.vector.tensor_copy(out=outp(5), in_=cbz)
                frames = [e1x, e1y, e1z, e2x, e2y, e2z, e3x, e3y, e3z]
                notv = bt(5)  # reuse
                nc.vector.tensor_scalar(notv, valid, -1.0, 1.0, A.mult, A.add)
                for j in range(3):
                    for i in range(3):
                        src = frames[j * 3 + i]
                        dst = outp(6 + i * 3 + j)
                        nc.vector.tensor_tensor(out=dst, in0=src, in1=valid,
                                                op=A.mult)
                        if i == j:
                            nc.vector.tensor_tensor(out=dst, in0=dst, in1=notv,
                                                    op=A.add)
                nc.vector.tensor_copy(out=outp(15), in_=mask)

                OutS = bulk_pool.tile([P, NWIN * 16], f32)
                nc.vector.tensor_copy(
                    out=OutS[:].rearrange("p (w c) -> p c w", c=16),
                    in_=OutP[:].rearrange("p (c w) -> p c w", w=NWIN))

                nc.sync.dma_start(
                    out=out_feat[:].rearrange("(w p) f -> p w f", p=P),
                    in_=stage_feat[:].rearrange("p (w f) -> p w f", f=HID))
                nc.sync.dma_start(
                    out=out_small[:].rearrange("(w p) c -> p w c", p=P),
                    in_=OutS[:].rearrange("p (w c) -> p w c", c=16))
    nc.finalize()
    return nc


def _host_prep(node_features, node_positions, atom_type, atom2residue):
    seg = np.asarray(atom2residue, dtype=np.int64)
    atype = np.asarray(atom_type, dtype=np.int64)
    pos = np.asarray(node_positions, dtype=np.float32)
    r_edges = [round(k * R_TOT / NCORES) for k in range(NCORES + 1)]
    a_edges = np.searchsorted(seg, r_edges).astype(np.int64)
    a_edges[0], a_edges[-1] = 0, A_TOT

    in_maps, metas = [], []
    for k in range(NCORES):
        a0, a1 = int(a_edges[k]), int(a_edges[k + 1])
        assert a1 - a0 <= APAD, f"core {k}: {a1 - a0} atoms > APAD"
        s = min(a0, A_TOT - APAD)
        sl = slice(s, s + APAD)
        segk = seg[sl]
        typk = atype[sl]
        posk = pos[sl]
        off = np.arange(APAD)
        real = (off >= a0 - s) & (off < a1 - s)

        segr = np.where(real, segk, -1)
        uids, first = np.unique(segr, return_index=True)
        if uids[0] == -1:
            uids, first = uids[1:], first[1:]
        win = first // WATOMS
        rank = np.zeros_like(win)
        for w in np.unique(win):
            m = win == w
            nw = int(m.sum())
            assert nw <= P, f"core {k} window {w}: {nw} residues > 128"
            rank[m] = np.arange(nw)
        vid = win * P + rank
        vmap = np.full(R_TOT, -1, dtype=np.int64)
        vmap[uids] = vid
        segv = np.where(real, vmap[np.clip(segr, 0, R_TOT - 1)], -1)

        ca = real & (typk == ID_CA)
        cb = real & (typk == ID_CB)
        isn = real & (typk == ID_N)
        isc = real & (typk == ID_C)

        def tele(mask):
            d = np.zeros((APAD, 3), np.float32)
            idx = np.nonzero(mask)[0]
            if len(idx):
                same = np.zeros(len(idx), bool)
                same[1:] = segk[idx[1:]] == segk[idx[:-1]]
                prev = np.zeros((len(idx), 3), np.float32)
                prev[1:] = posk[idx[:-1]]
                d[idx] = posk[idx] - np.where(same[:, None], prev, 0.0)
            return d

        dca = tele(ca)
        dcb = tele(cb)

        cols = np.zeros((APAD, 16), np.float32)
        segv_f = np.where(segv >= 0, segv, -100000).astype(np.float32)
        cols[:, 0] = np.where(ca, segv_f, -100000.0)   # CA-masked seg
        cols[:, 1] = segv_f                            # unmasked seg
        cols[:, 2] = ca
        cols[:, 3] = real.astype(np.float32) + ENC * isn
        cols[:, 4] = isc.astype(np.float32) + ENC * cb
        cols[:, 5:8] = dca
        cols[:, 8:11] = dcb
        cols_sw = np.ascontiguousarray(
            cols.reshape(NCHUNK, TPC, P, 16).transpose(0, 2, 1, 3))

        in_maps.append({
            "feats": np.ascontiguousarray(node_features[sl]),
            "cols": cols_sw,
        })
        metas.append({"uids": uids, "vid": vid})
    return in_maps, metas


def kernel(node_features, node_positions, atom_type, atom2residue,
           num_residues=R_TOT):
    node_features = np.asarray(node_features, dtype=np.float32)
    node_positions = np.asarray(node_positions, dtype=np.float32)
    atom_type_np = np.asarray(atom_type, dtype=np.int32)
    atom2residue_np = np.asarray(atom2residue, dtype=np.int32)

    in_maps, metas = _host_prep(node_features, node_positions,
                                atom_type_np, atom2residue_np)
    if "nc" not in _COMPILED:
        _COMPILED["nc"] = _build_nc()
    nc = _COMPILED["nc"]

    trace = bool(int(os.environ.get("KERNEL_TRACE", "0")))
    res = run_bass_kernel_spmd(nc, in_maps, core_ids=list(range(NCORES)),
                               trace=trace)
    if res.exec_time_ns is not None:
        print(f"HW exec time: {res.exec_time_ns} ns")
        _COMPILED["exec_time_ns"] = res.exec_time_ns

    R = R_TOT
    res_feat = np.zeros((R, HID), np.float32)
    pos_ca = np.zeros((R, 3), np.float32)
    pos_cb = np.zeros((R, 3), np.float32)
    frames = np.tile(np.eye(3, dtype=np.float32)[None], (R, 1, 1))
    maskf = np.zeros(R, np.float32)
    for k in range(NCORES):
        m = metas[k]
        out_f = res.results[k]["out_feat"]
        out_s = res.results[k]["out_small"]
        uids, vid = m["uids"], m["vid"]
        res_feat[uids] = out_f[vid]
        sm = out_s[vid]
        pos_ca[uids] = sm[:, 0:3]
        pos_cb[uids] = sm[:, 3:6]
        frames[uids] = sm[:, 6:15].reshape(-1, 3, 3)
        maskf[uids] = sm[:, 15]
    frames[R - 1] = np.eye(3, dtype=np.float32)
    residue_mask = maskf > 0.5
    return (res_feat, pos_ca, pos_cb, frames, atom2residue_np, residue_mask)
